# revision 50
# baseline (speedup 1.0000x reference)
"""Causal self-attention (B=2, T=2048, C=1024, H=16, Dh=64) on 8 TRN2 cores.

Sharding: data-parallel over B (2) x tensor-parallel over heads (4 groups of
4 heads) = 8 shards. Core i handles batch i//4, heads 4*(i%4)..4*(i%4)+3.
Host pre-marshals each shard's operands (slice + transpose to contraction-
major + cast to bf16); each core computes its QKV projection, causal-softmax
attention for its 4 heads, and its partial out-projection (bf16). Host sums
the 4 partials per batch (row-parallel out-projection reduce).

Device program (per core, all matmuls bf16 with f32 PSUM accumulation):
  xt  [1024, 2048] bf16 = x[b].T
  wt  [1024, 768]  bf16 = Wqkv_shard.T, cols = [Q0|K0|Vp0|Q1|K1|Vp1] x 128
  wot [256, 1024]  bf16 = Wout[:, cols].T
  y   [2048, 1024] bf16 partial output

Schedule: PE is the global bottleneck (~113us of matmul columns at 2.4GHz).
Keep it gapless: pre-ramp dummy matmuls during the DMA lead-in, a per-chunk
interleaved prefix (Q0,K0,V0..3 per k-chunk) paced to DMA arrival, then two
ACT-paced attention windows (head pairs 0 and 1) with all remaining
projection/out-projection work injected as fine-grained PE filler, reserves
ordered so the pair-1 window and block boundaries never starve.  Diagonal
kts get extra filler (their score/PV columns shrink while exp cost doesn't).
The final window's 1/Z chain (PSUM Z row -> DMA transpose -> bf16
reciprocal -> DMA back -> ones-matmul broadcast) runs as two independent
half-chains on separate queues while the PE drains the remaining filler
reserve and the freed-bank wide out-projections; the last two y tiles
split their eviction copies/DMAs per 512-col half across engines/queues to
shorten the output drain.

Scheduling invariant learned the hard way: total span is minimized iff the
PE never idles -- moving work between windows and bursts is otherwise
neutral, so keep every window's filler supply >= its ACT-pace deficit.
Run-to-run HW variance is +-1.5us (free-running HAM phase, P-state).
"""

import sys

for _p in ("/opt/trn_rl_repo",):
    if _p not in sys.path:
        sys.path.append(_p)

import numpy as np
import ml_dtypes
from contextlib import ExitStack

import concourse.bass as bass
import concourse.bacc as bacc
import concourse.mybir as mybir
import concourse.tile as tile
from concourse.bass_utils import run_bass_kernel_spmd
from concourse.masks import make_upper_triangular

BF16 = mybir.dt.bfloat16
F32 = mybir.dt.float32
AF = mybir.ActivationFunctionType

T = 2048
C = 1024
N_CORES = 8

_cached_nc = None


def build_program():
    global _cached_nc
    if _cached_nc is not None:
        return _cached_nc
    nc = bacc.Bacc("TRN2", target_bir_lowering=False, debug=False,
                   num_devices=N_CORES)
    xt_d = nc.dram_tensor("xt", [C, T], BF16, kind="ExternalInput").ap()
    wt_d = nc.dram_tensor("wt", [C, 768], BF16, kind="ExternalInput").ap()
    wot_d = nc.dram_tensor("wot", [256, C], BF16, kind="ExternalInput").ap()
    y_d = nc.dram_tensor("y", [T, C], BF16, kind="ExternalOutput").ap()

    with tile.TileContext(nc) as tc, ExitStack() as ctx:
        const = ctx.enter_context(tc.tile_pool(name="const", bufs=1))
        sb = ctx.enter_context(tc.tile_pool(name="sb", bufs=1))
        wk = ctx.enter_context(tc.tile_pool(name="wk", bufs=1))
        ps = ctx.enter_context(tc.tile_pool(name="ps", bufs=1, space="PSUM"))

        trimask = const.tile([128, 128], BF16, tag="trimask")
        make_upper_triangular(nc, trimask[:], val=1.0, diag=True)
        zbias = const.tile([128, 1], F32, tag="zbias")
        nc.vector.memset(zbias[:], 0.0)
        dmy = const.tile([128, 256], BF16, tag="dmy")
        nc.vector.memset(dmy[:], 0.0)
        onesb = const.tile([1, 64], BF16, tag="onesb")
        nc.vector.memset(onesb[:], 1.0)
        # preload the ACT exp table during the DMA lead-in
        warm = const.tile([128, 1], BF16, tag="warm")
        nc.scalar.activation(warm[:], zbias[:], AF.Exp, bias=zbias[:],
                             scale=1.0)

        XTW = [sb.tile([128, 2048], BF16, tag=f"xt{k}", name=f"xts{k}")
               for k in range(8)]
        WT = [sb.tile([128, 768], BF16, tag=f"wt{k}", name=f"wts{k}")
              for k in range(8)]
        WOT = [sb.tile([128, C], BF16, tag=f"wot{k}", name=f"wots{k}")
               for k in range(2)]
        QT = [sb.tile([128, T], BF16, tag=f"qt{p}", name=f"qts{p}")
              for p in range(2)]
        KT = [sb.tile([128, T], BF16, tag=f"kt{p}", name=f"kts{p}")
              for p in range(2)]
        V = [sb.tile([128, 4 * 65], BF16, tag=f"v{t}", name=f"vs{t}")
             for t in range(16)]
        OUTT = [sb.tile([128, T], BF16, tag=f"outt{p}", name=f"outts{p}")
                for p in range(2)]

        # DMA queues are descriptor-rate bound (1 descriptor per partition
        # row), so balance DESCRIPTOR load across all 3 HWDGE queues in
        # need-order: compute-enabling prefix (WT piece [Q0|K0|Vp0] + xt
        # nb0) first, then WT piece 2, xt nb1, xt nb2+3 (one wide
        # transfer), wot last (first use is mid pair-1 window)
        # NEVER put DMA triggers on the scalar queue: they serialize ahead
        # of the exp stream on the ACT sequencer and stall attention.
        # Weights + y ride the gpsimd queue, xt + the Z-transpose chain
        # ride sync.
        for k in range(8):
            nc.gpsimd.dma_start(WT[k][:, 0:384],
                                wt_d[128 * k:128 * (k + 1), 0:384])
            nc.sync.dma_start(
                XTW[k][:, 0:512], xt_d[128 * k:128 * (k + 1), 0:512])
        for k in range(8):
            nc.gpsimd.dma_start(
                WT[k][:, 384:768], wt_d[128 * k:128 * (k + 1), 384:768])
        for k in range(8):
            nc.sync.dma_start(
                XTW[k][:, 512:1024], xt_d[128 * k:128 * (k + 1), 512:1024])
        for k in range(8):
            nc.sync.dma_start(
                XTW[k][:, 1024:2048], xt_d[128 * k:128 * (k + 1), 1024:2048])
        for k in range(2):
            nc.gpsimd.dma_start(WOT[k][:], wot_d[128 * k:128 * (k + 1), :])

        # PSUM budget (8 banks): "st" [128,1024]x2 = 4 (attention scores),
        # "pv" [128,1024]x1 = 2 (attention out), "pj" [128,512]x2 = 2
        # (projections + out-projection).

        class Filler:
            """Queue of matmul-emission steps; fill(n) emits up to n matmuls
            of PE filler work (psum accumulation groups + their evictions)
            inside ACT-paced attention windows."""

            def __init__(self):
                self.steps = []   # flat list of callables, each emits 1 MM
                self.pos = 0

            def add_group(self, mk_steps):
                self.steps.extend(mk_steps)
                return len(self.steps)   # marker: position after this group

            def fill(self, n):
                end = min(self.pos + n, len(self.steps))
                while self.pos < end:
                    self.steps[self.pos]()
                    self.pos += 1

            def fill_until(self, marker):
                self.fill(max(0, marker - self.pos))

            def drain(self):
                self.fill(len(self.steps))

        def proj_qk_steps(p, which, nb):
            dst = QT[p] if which == 0 else KT[p]
            fb = 384 * p + 128 * which
            box = {}

            def step(kc):
                def go():
                    if kc == 0:
                        box['pj'] = ps.tile([128, 512], F32, tag="pj",
                                            bufs=2, name="pj")
                    nc.tensor.matmul(
                        box['pj'][:],
                        WT[kc][:, fb:fb + 128],
                        XTW[kc][:, nb * 512:(nb + 1) * 512],
                        start=(kc == 0), stop=(kc == 7))
                    if kc == 7:
                        nc.vector.tensor_copy(
                            dst[:, nb * 512:(nb + 1) * 512], box['pj'][:])
                return go
            return [step(kc) for kc in range(8)]

        def proj_v_steps(tt, pp):
            """V projection for token tile tt, head-pair pp (128 cols)."""
            vbase = 256 + 384 * pp
            box = {}

            def step(kc):
                def go():
                    if kc == 0:
                        box['pj'] = ps.tile([128, 512], F32, tag="pj",
                                            bufs=2, name="pj")
                    nc.tensor.matmul(
                        box['pj'][:, 0:128],
                        XTW[kc][:, tt * 128:(tt + 1) * 128],
                        WT[kc][:, vbase:vbase + 128],
                        start=(kc == 0), stop=(kc == 7))
                    if kc == 7:
                        pj = box['pj']
                        vv = V[tt].rearrange("p (h e) -> p h e", e=65)
                        nc.vector.tensor_copy(
                            vv[:, 2 * pp:2 * pp + 2, 0:64],
                            pj[:, 0:128].rearrange("p (h e) -> p h e", e=64))
                        nc.vector.memset(vv[:, 2 * pp:2 * pp + 2, 64:65], 1.0)
                return go
            return [step(kc) for kc in range(8)]

        def outproj_steps(tt):
            box = {}

            def step(fb, kcp):
                def go():
                    if fb == 0 and kcp == 0:
                        box['ysb'] = wk.tile([128, C], BF16, tag="ysb",
                                             bufs=2, name="ysb")
                    if kcp == 0:
                        box[fb] = ps.tile([128, 512], F32, tag="pj",
                                          bufs=2, name="pj")
                    nc.tensor.matmul(
                        box[fb][:],
                        OUTT[kcp][:, tt * 128:(tt + 1) * 128],
                        WOT[kcp][:, fb * 512:(fb + 1) * 512],
                        start=(kcp == 0), stop=(kcp == 1))
                    if kcp == 1:
                        nc.vector.tensor_copy(
                            box['ysb'][:, fb * 512:(fb + 1) * 512],
                            box[fb][:])
                    if fb == 1 and kcp == 1:
                        eng = nc.sync if tt % 2 else nc.gpsimd
                        eng.dma_start(y_d[tt * 128:(tt + 1) * 128, :],
                                      box['ysb'][:])
                return go
            return [step(fb, kcp) for fb in range(2) for kcp in range(2)]

        # Wide out-projection for the tail: once the attention PSUM banks
        # free up, each token tile gets a full [128,1024] tile (both fb
        # halves, no pj-buffer contention), the OUTT[0] contraction half
        # runs before the last normalize lands, and y DMAs alternate
        # queues.
        wbox = {}

        def wide_pre(tt):
            tag = "pv" if tt in (10, 13) else "st"
            w = ps.tile([128, 1024], F32, tag=tag,
                        bufs=(1 if tag == "pv" else 2), name=f"w{tt}")
            if tt >= 14:
                # separate half tiles: the ACT and DVE eviction copies run
                # concurrently (same-tile writes serialize across engines)
                ysb = (wk.tile([128, 512], BF16, tag="ysbwL", bufs=2,
                               name=f"ywL{tt}"),
                       wk.tile([128, 512], BF16, tag="ysbwR", bufs=2,
                               name=f"ywR{tt}"))
            else:
                ysb = wk.tile([128, C], BF16, tag="ysbw", bufs=4,
                              name=f"yw{tt}")
            wbox[tt] = (w, ysb)
            for fb in range(2):
                nc.tensor.matmul(
                    w[:, fb * 512:(fb + 1) * 512],
                    OUTT[0][:, tt * 128:(tt + 1) * 128],
                    WOT[0][:, fb * 512:(fb + 1) * 512],
                    start=True, stop=False)

        def wide_post(tt, split=False):
            w, ysb = wbox[tt]
            for fb in range(2):
                nc.tensor.matmul(
                    w[:, fb * 512:(fb + 1) * 512],
                    OUTT[1][:, tt * 128:(tt + 1) * 128],
                    WOT[1][:, fb * 512:(fb + 1) * 512],
                    start=False, stop=True)
            if isinstance(ysb, tuple):
                ysbL, ysbR = ysb
                nc.scalar.copy(ysbL[:], w[:, 0:512])
                nc.sync.dma_start(y_d[tt * 128:(tt + 1) * 128, 0:512],
                                  ysbL[:])
                nc.vector.tensor_copy(ysbR[:], w[:, 512:1024])
                nc.gpsimd.dma_start(y_d[tt * 128:(tt + 1) * 128, 512:1024],
                                    ysbR[:])
            else:
                nc.scalar.copy(ysb[:, 0:512], w[:, 0:512])
                nc.vector.tensor_copy(ysb[:, 512:1024], w[:, 512:1024])
                eng = nc.sync if tt % 2 else nc.gpsimd
                eng.dma_start(y_d[tt * 128:(tt + 1) * 128, :], ysb[:])

        def attention_qb(p, qb, filler=None, rate=0, boundary=0, final=False,
                         adapt=False):
            hA, hB = 2 * p, 2 * p + 1
            if filler is not None and boundary:
                filler.fill(boundary)
            # merged A/B psum: head A in cols 0:512, head B in 512:1024
            pv = ps.tile([128, 1024], F32, tag="pv", bufs=1, name="pv")
            nkt = (qb + 1) * 4

            def emit_pv(kt, sa, off, ncols):
                nc.tensor.matmul(
                    pv[0:65, off:512],
                    V[kt][:, hA * 65:hA * 65 + 65],
                    sa[:, 0:ncols],
                    start=(kt == 0), stop=(kt == nkt - 1))
                nc.tensor.matmul(
                    pv[0:65, 512 + off:1024],
                    V[kt][:, hB * 65:hB * 65 + 65],
                    sa[:, 512:512 + ncols],
                    start=(kt == 0), stop=(kt == nkt - 1))

            # software-pipelined: PV(kt) is emitted after ST(kt+1) so the
            # in-order PE never waits on the exp of the current iteration
            pending = None
            for kt in range(nkt):
                off = max(0, kt * 128 - qb * 512)
                ncols = 512 - off
                qs = qb * 512 + off
                stp = ps.tile([128, 1024], F32, tag="st", bufs=2, name="stp")
                nc.tensor.matmul(
                    stp[:, 0:ncols],
                    KT[p][0:64, kt * 128:(kt + 1) * 128],
                    QT[p][0:64, qs:qs + ncols],
                    start=True, stop=True)
                nc.tensor.matmul(
                    stp[:, 512:512 + ncols],
                    KT[p][64:128, kt * 128:(kt + 1) * 128],
                    QT[p][64:128, qs:qs + ncols],
                    start=True, stop=True)
                sa = wk.tile([128, 1024], BF16, tag="sa_sb", bufs=3, name="sa")
                nc.scalar.activation(
                    sa.rearrange("p (g n) -> p g n", g=2)[:, :, 0:ncols],
                    stp.rearrange("p (g n) -> p g n", g=2)[:, :, 0:ncols],
                    AF.Exp, bias=zbias[:], scale=0.125)
                if off > 0 or kt * 128 == qb * 512:
                    m3 = sa.rearrange("p (g n) -> p g n", g=2)[:, :, 0:128]
                    nc.vector.tensor_mul(
                        m3, m3,
                        trimask[:].unsqueeze(1).broadcast_to([128, 2, 128]))
                if filler is not None:
                    extra = (0 if ncols == 512 else
                             (1 if ncols >= 256 else 2))
                    filler.fill(rate + extra)
                if pending is not None:
                    emit_pv(*pending)
                pending = (kt, sa, off, ncols)
            emit_pv(*pending)
            # evict unnormalized out + Z; 1/Z via DMA-reshape so the
            # reciprocal runs on 128 DVE lanes instead of one
            if final:
                zr = wk.tile([1, 1024], F32, tag="zr", bufs=1, name="zr")
                nc.vector.tensor_copy(zr[:, 0:512], pv[64:65, 0:512])
                nc.scalar.copy(zr[:, 512:1024], pv[64:65, 512:1024])
            u = wk.tile([65, 1024], F32, tag="u", bufs=2, name="u")
            nc.vector.tensor_copy(u[:, 0:512], pv[0:65, 0:512])
            # second half on ACT: halves the eviction latency and keeps DVE
            # free for the mask/normalize chain feeding the next window
            nc.scalar.copy(u[:, 512:1024], pv[0:65, 512:1024])
            qsl = slice(qb * 512, (qb + 1) * 512)
            if final:
                # transposed 1/Z (reciprocal across DVE lanes) as two
                # independent half-chains on separate DMA queues, then two
                # 1-deep matmuls against a ones row replicate it across
                # partitions
                zcA = wk.tile([64, 8], F32, tag="zcA", bufs=1, name="zcA")
                zcB = wk.tile([64, 8], F32, tag="zcB", bufs=1, name="zcB")
                nc.sync.dma_start(zcA[:], zr[:, 0:512])
                nc.gpsimd.dma_start(zcB[:], zr[:, 512:1024])
                zcbA = wk.tile([64, 8], BF16, tag="zcbA", bufs=1, name="zcbA")
                zcbB = wk.tile([64, 8], BF16, tag="zcbB", bufs=1, name="zcbB")
                with nc.allow_low_precision(reason="bf16 1/Z, ~0.4% rel"):
                    nc.vector.reciprocal(zcbA[:], zcA[:])
                    nc.vector.reciprocal(zcbB[:], zcB[:])
                zrbA = wk.tile([1, 512], BF16, tag="zrbA", bufs=1, name="zrbA")
                zrbB = wk.tile([1, 512], BF16, tag="zrbB", bufs=1, name="zrbB")
                nc.sync.dma_start(zrbA[:], zcbA[:])
                nc.gpsimd.dma_start(zrbB[:], zcbB[:])
                # the chain above runs on DVE/ACT/sync/gpsimd; the PE
                # meanwhile drains the remaining filler reserve (qb<=1
                # out-projections) and runs the freed-bank wide
                # out-projections for qb2's tokens + the OUTT[0] halves of
                # qb3's
                if filler is not None:
                    filler.drain()
                for tt in (8, 9, 10, 11):
                    wide_pre(tt)
                    wide_post(tt)
                wide_pre(12)
                wide_pre(13)
                wide_pre(14)
                zpA = ps.tile([128, 512], F32, tag="pj", bufs=2, name="zpA")
                zpB = ps.tile([128, 512], F32, tag="pj", bufs=2, name="zpB")
                nc.tensor.matmul(zpA[0:64, :], onesb[:], zrbA[:],
                                 start=True, stop=True)
                nc.vector.tensor_mul(OUTT[p][0:64, qsl], u[0:64, 0:512],
                                     zpA[0:64, :])
                nc.tensor.matmul(zpB[0:64, :], onesb[:], zrbB[:],
                                 start=True, stop=True)
                nc.vector.tensor_mul(OUTT[p][64:128, qsl], u[0:64, 512:1024],
                                     zpB[0:64, :])
                wide_post(12)
                wide_post(13)
                wide_pre(15)
                wide_post(14, split=True)
                wide_post(15, split=True)
            else:
                zcol = wk.tile([128, 8], F32, tag="zcol", bufs=2, name="zcol")
                nc.sync.dma_start(zcol[:], u[64:65, :])
                nc.vector.reciprocal(zcol[:], zcol[:])
                zrow = wk.tile([1, 1024], F32, tag="zrow", bufs=2, name="zrow")
                nc.sync.dma_start(zrow[:], zcol[:])
                zb = wk.tile([64, 1024], F32, tag="zb", bufs=2, name="zb")
                nc.gpsimd.partition_broadcast(zb[:], zrow[:])
                nc.vector.tensor_mul(OUTT[p][0:64, qsl], u[0:64, 0:512],
                                     zb[:, 0:512])
                nc.vector.tensor_mul(OUTT[p][64:128, qsl], u[0:64, 512:1024],
                                     zb[:, 512:1024])

        # ---- emission order = PE order (in-order engine) ----
        # Pre-ramp: dummy matmuls with no data deps keep the PE busy from
        # the end of the framework preamble so it reaches full p-state
        # before real data lands, and pad the DMA-paced prefix so arrival
        # gaps never reset the ramp.
        stdum = ps.tile([128, 1024], F32, tag="st", bufs=2, name="stdum")

        def dummy_mm():
            nc.tensor.matmul(stdum[:, 0:256], dmy[:, 0:128], dmy[:],
                             start=True, stop=True, skip_group_check=True)

        for _ in range(8):
            dummy_mm()

        # Prefix phase A, paced to DMA: per k-chunk [Q0, K0, dummy, dummy]
        # so each 224KB chunk arrival feeds ~1.7us of PE work at mid
        # p-state. One accumulation group per PSUM bank (the PE start flag
        # zeroes bank-granular regions).
        pjQ = ps.tile([128, 512], F32, tag="pj", bufs=2, name="pjQ")
        pjK = ps.tile([128, 512], F32, tag="pj", bufs=2, name="pjK")
        pjV = ps.tile([128, 1024], F32, tag="pv", bufs=1, name="pjV")
        stV = ps.tile([128, 1024], F32, tag="st", bufs=2, name="stV")
        for kc in range(8):
            nc.tensor.matmul(pjQ[:], WT[kc][:, 0:128],
                             XTW[kc][:, 0:512],
                             start=(kc == 0), stop=(kc == 7))
            nc.tensor.matmul(pjK[:], WT[kc][:, 128:256],
                             XTW[kc][:, 0:512],
                             start=(kc == 0), stop=(kc == 7))
            nc.tensor.matmul(pjV[:, 0:128],
                             XTW[kc][:, 0:128], WT[kc][:, 256:384],
                             start=(kc == 0), stop=(kc == 7))
            nc.tensor.matmul(pjV[:, 512:640],
                             XTW[kc][:, 128:256], WT[kc][:, 256:384],
                             start=(kc == 0), stop=(kc == 7))
            nc.tensor.matmul(stV[:, 0:128],
                             XTW[kc][:, 256:384], WT[kc][:, 256:384],
                             start=(kc == 0), stop=(kc == 7))
            nc.tensor.matmul(stV[:, 512:640],
                             XTW[kc][:, 384:512], WT[kc][:, 256:384],
                             start=(kc == 0), stop=(kc == 7))
        nc.vector.tensor_copy(QT[0][:, 0:512], pjQ[:])
        nc.vector.tensor_copy(KT[0][:, 0:512], pjK[:])
        vdst = {0: pjV[:, 0:128], 1: pjV[:, 512:640],
                2: stV[:, 0:128], 3: stV[:, 512:640]}
        for tt in (0, 1, 2, 3):
            vv = V[tt].rearrange("p (h e) -> p h e", e=65)
            nc.vector.tensor_copy(
                vv[:, 0:2, 0:64],
                vdst[tt].rearrange("p (h e) -> p h e", e=64))
            nc.vector.memset(vv[:, 0:2, 64:65], 1.0)
        # release the dummy psum tile
        scratch = wk.tile([128, 8], F32, tag="scratch", bufs=1, name="scratch")
        nc.vector.tensor_copy(scratch[:], stdum[:, 0:8])

        # fillA: lead with work whose inputs are already resident (Q1/K1
        # nb0 needs only WT piece 2), then pair-0's deps in qb order, then
        # pair-1 early work
        fillA = Filler()
        markA = {}
        for w in range(2):
            fillA.add_group(proj_qk_steps(1, w, 0))
        for nb in (1, 2, 3):
            for w in range(2):
                fillA.add_group(proj_qk_steps(0, w, nb))
            m = 0
            for tt in range(4 * nb, 4 * nb + 4):
                m = fillA.add_group(proj_v_steps(tt, 0))
            markA[nb] = m
        for w in range(2):
            fillA.add_group(proj_qk_steps(1, w, 1))
        for tt in range(0, 8):
            fillA.add_group(proj_v_steps(tt, 1))

        for qb in range(4):
            if qb in markA:
                fillA.fill_until(markA[qb])
            attention_qb(0, qb, filler=fillA, rate=3,
                         boundary=(8 if qb >= 2 else 6 if qb else 0))
        fillA.drain()

        # fillB: pair-1 reserves ordered so every boundary has ready work,
        # out-projection groups one block late so OUTT[1] normalize chains
        # complete off-engine before their matmuls are reached
        fillB = Filler()
        for w in range(2):
            fillB.add_group(proj_qk_steps(1, w, 2))
        mB2 = 0
        for tt in range(8, 12):
            mB2 = fillB.add_group(proj_v_steps(tt, 1))
        for tt in range(0, 4):
            fillB.add_group(outproj_steps(tt))
        for w in range(2):
            fillB.add_group(proj_qk_steps(1, w, 3))
        mB3 = 0
        for tt in range(12, 16):
            mB3 = fillB.add_group(proj_v_steps(tt, 1))
        for tt in range(4, 8):
            fillB.add_group(outproj_steps(tt))

        attention_qb(1, 0, filler=fillB, rate=2, boundary=6)
        attention_qb(1, 1, filler=fillB, rate=2, boundary=6)
        fillB.fill_until(mB2)
        attention_qb(1, 2, filler=fillB, rate=3, boundary=6)
        fillB.fill_until(mB3)
        attention_qb(1, 3, filler=fillB, rate=1, boundary=4, final=True)
        fillB.drain()

    nc.compile()
    _cached_nc = nc
    return nc


def shard_inputs(x, Wqkv, Wout):
    """Full inputs -> 8 per-core input dicts (sliced/transposed/bf16-cast)."""
    bf = ml_dtypes.bfloat16
    Wq, Wk, Wv = Wqkv[0:1024], Wqkv[1024:2048], Wqkv[2048:3072]
    in_maps = []
    for i in range(N_CORES):
        b, g = divmod(i, 4)
        r0 = slice(256 * g, 256 * g + 128)
        r1 = slice(256 * g + 128, 256 * (g + 1))
        w_my = np.concatenate(
            [Wq[r0], Wk[r0], Wv[r0], Wq[r1], Wk[r1], Wv[r1]], axis=0)
        r = slice(256 * g, 256 * (g + 1))
        in_maps.append({
            "xt": np.ascontiguousarray(x[b].T).astype(bf),
            "wt": np.ascontiguousarray(w_my.T).astype(bf),
            "wot": np.ascontiguousarray(Wout[:, r].T).astype(bf),
        })
    return in_maps


def gather_output(results):
    """8 per-core partial y (bf16) -> full [2, T, C] f32 output."""
    y = np.zeros((2, T, C), dtype=np.float64)
    for i in range(N_CORES):
        y[i // 4] += np.asarray(results[i]["y"], dtype=np.float64)
    return y.astype(np.float32)


def kernel(x, Wqkv, Wout):
    x = np.asarray(x)
    Wqkv = np.asarray(Wqkv)
    Wout = np.asarray(Wout)
    nc = build_program()
    in_maps = shard_inputs(x, Wqkv, Wout)
    res = run_bass_kernel_spmd(nc, in_maps, core_ids=list(range(N_CORES)))
    return gather_output(res.results)



# revision 51
# speedup vs baseline: 1.0056x; 1.0056x over previous
"""Causal self-attention (B=2, T=2048, C=1024, H=16, Dh=64) on 8 TRN2 cores.

Sharding: data-parallel over B (2) x tensor-parallel over heads (4 groups of
4 heads) = 8 shards. Core i handles batch i//4, heads 4*(i%4)..4*(i%4)+3.
Host pre-marshals each shard's operands (slice + transpose to contraction-
major + cast to bf16); each core computes its QKV projection, causal-softmax
attention for its 4 heads, and its partial out-projection (bf16). Host sums
the 4 partials per batch (row-parallel out-projection reduce).

Device program (per core, all matmuls bf16 with f32 PSUM accumulation):
  xt  [1024, 2048] bf16 = x[b].T
  wt  [1024, 768]  bf16 = Wqkv_shard.T, cols = [Q0|K0|Vp0|Q1|K1|Vp1] x 128
  wot [256, 1024]  bf16 = Wout[:, cols].T
  y   [2048, 1024] bf16 partial output

Schedule: PE is the global bottleneck (~113us of matmul columns at 2.4GHz).
Keep it gapless: pre-ramp dummy matmuls during the DMA lead-in, a per-chunk
interleaved prefix (Q0,K0,V0..3 per k-chunk) paced to DMA arrival, then two
ACT-paced attention windows (head pairs 0 and 1) with all remaining
projection/out-projection work injected as fine-grained PE filler, reserves
ordered so the pair-1 window and block boundaries never starve.  Diagonal
kts get extra filler (their score/PV columns shrink while exp cost doesn't).
The final window's 1/Z chain (PSUM Z row -> DMA transpose -> bf16
reciprocal -> DMA back -> ones-matmul broadcast) runs as two independent
half-chains on separate queues while the PE drains the remaining filler
reserve and the freed-bank wide out-projections; the last two y tiles
split their eviction copies/DMAs per 512-col half across engines/queues to
shorten the output drain.

Scheduling invariant learned the hard way: total span is minimized iff the
PE never idles -- moving work between windows and bursts is otherwise
neutral, so keep every window's filler supply >= its ACT-pace deficit.
Run-to-run HW variance is +-1.5us (free-running HAM phase, P-state).
"""

import sys

for _p in ("/opt/trn_rl_repo",):
    if _p not in sys.path:
        sys.path.append(_p)

import numpy as np
import ml_dtypes
from contextlib import ExitStack

import concourse.bass as bass
import concourse.bacc as bacc
import concourse.mybir as mybir
import concourse.tile as tile
from concourse.bass_utils import run_bass_kernel_spmd
from concourse.masks import make_upper_triangular

BF16 = mybir.dt.bfloat16
F32 = mybir.dt.float32
AF = mybir.ActivationFunctionType

T = 2048
C = 1024
N_CORES = 8

_cached_nc = None


def build_program():
    global _cached_nc
    if _cached_nc is not None:
        return _cached_nc
    nc = bacc.Bacc("TRN2", target_bir_lowering=False, debug=False,
                   num_devices=N_CORES)
    xt_d = nc.dram_tensor("xt", [C, T], BF16, kind="ExternalInput").ap()
    wt_d = nc.dram_tensor("wt", [C, 768], BF16, kind="ExternalInput").ap()
    wot_d = nc.dram_tensor("wot", [256, C], BF16, kind="ExternalInput").ap()
    y_d = nc.dram_tensor("y", [T, C], BF16, kind="ExternalOutput").ap()

    with tile.TileContext(nc) as tc, ExitStack() as ctx:
        const = ctx.enter_context(tc.tile_pool(name="const", bufs=1))
        sb = ctx.enter_context(tc.tile_pool(name="sb", bufs=1))
        wk = ctx.enter_context(tc.tile_pool(name="wk", bufs=1))
        ps = ctx.enter_context(tc.tile_pool(name="ps", bufs=1, space="PSUM"))

        trimask = const.tile([128, 128], BF16, tag="trimask")
        make_upper_triangular(nc, trimask[:], val=1.0, diag=True)
        zbias = const.tile([128, 1], F32, tag="zbias")
        nc.vector.memset(zbias[:], 0.0)
        dmy = const.tile([128, 256], BF16, tag="dmy")
        nc.vector.memset(dmy[:], 0.0)
        onesb = const.tile([1, 64], BF16, tag="onesb")
        nc.vector.memset(onesb[:], 1.0)
        # preload the ACT exp table during the DMA lead-in
        warm = const.tile([128, 1], BF16, tag="warm")
        nc.scalar.activation(warm[:], zbias[:], AF.Exp, bias=zbias[:],
                             scale=1.0)

        XTW = [sb.tile([128, 2048], BF16, tag=f"xt{k}", name=f"xts{k}")
               for k in range(8)]
        WT = [sb.tile([128, 768], BF16, tag=f"wt{k}", name=f"wts{k}")
              for k in range(8)]
        WOT = [sb.tile([128, C], BF16, tag=f"wot{k}", name=f"wots{k}")
               for k in range(2)]
        QT = [sb.tile([128, T], BF16, tag=f"qt{p}", name=f"qts{p}")
              for p in range(2)]
        KT = [sb.tile([128, T], BF16, tag=f"kt{p}", name=f"kts{p}")
              for p in range(2)]
        V = [sb.tile([128, 4 * 65], BF16, tag=f"v{t}", name=f"vs{t}")
             for t in range(16)]
        OUTT = [sb.tile([128, T], BF16, tag=f"outt{p}", name=f"outts{p}")
                for p in range(2)]

        # DMA queues are descriptor-rate bound (1 descriptor per partition
        # row), so balance DESCRIPTOR load across all 3 HWDGE queues in
        # need-order: compute-enabling prefix (WT piece [Q0|K0|Vp0] + xt
        # nb0) first, then WT piece 2, xt nb1, xt nb2+3 (one wide
        # transfer), wot last (first use is mid pair-1 window)
        # NEVER put DMA triggers on the scalar queue: they serialize ahead
        # of the exp stream on the ACT sequencer and stall attention.
        # Weights + y ride the gpsimd queue, xt + the Z-transpose chain
        # ride sync.
        for k in range(8):
            nc.gpsimd.dma_start(WT[k][:, 0:384],
                                wt_d[128 * k:128 * (k + 1), 0:384])
            nc.sync.dma_start(
                XTW[k][:, 0:512], xt_d[128 * k:128 * (k + 1), 0:512])
        for k in range(8):
            nc.gpsimd.dma_start(
                WT[k][:, 384:768], wt_d[128 * k:128 * (k + 1), 384:768])
        for k in range(8):
            nc.sync.dma_start(
                XTW[k][:, 512:1024], xt_d[128 * k:128 * (k + 1), 512:1024])
        for k in range(8):
            nc.sync.dma_start(
                XTW[k][:, 1024:2048], xt_d[128 * k:128 * (k + 1), 1024:2048])
        for k in range(2):
            nc.gpsimd.dma_start(WOT[k][:], wot_d[128 * k:128 * (k + 1), :])

        # PSUM budget (8 banks): "st" [128,1024]x2 = 4 (attention scores),
        # "pv" [128,1024]x1 = 2 (attention out), "pj" [128,512]x2 = 2
        # (projections + out-projection).

        class Filler:
            """Queue of matmul-emission steps; fill(n) emits up to n matmuls
            of PE filler work (psum accumulation groups + their evictions)
            inside ACT-paced attention windows."""

            def __init__(self):
                self.steps = []   # flat list of callables, each emits 1 MM
                self.pos = 0

            def add_group(self, mk_steps):
                self.steps.extend(mk_steps)
                return len(self.steps)   # marker: position after this group

            def fill(self, n):
                end = min(self.pos + n, len(self.steps))
                while self.pos < end:
                    self.steps[self.pos]()
                    self.pos += 1

            def fill_until(self, marker):
                self.fill(max(0, marker - self.pos))

            def drain(self):
                self.fill(len(self.steps))

        def proj_qk_steps(p, which, nb):
            dst = QT[p] if which == 0 else KT[p]
            fb = 384 * p + 128 * which
            box = {}

            def step(kc):
                def go():
                    if kc == 0:
                        box['pj'] = ps.tile([128, 512], F32, tag="pj",
                                            bufs=2, name="pj")
                    nc.tensor.matmul(
                        box['pj'][:],
                        WT[kc][:, fb:fb + 128],
                        XTW[kc][:, nb * 512:(nb + 1) * 512],
                        start=(kc == 0), stop=(kc == 7))
                    if kc == 7:
                        nc.vector.tensor_copy(
                            dst[:, nb * 512:(nb + 1) * 512], box['pj'][:])
                return go
            return [step(kc) for kc in range(8)]

        def proj_v_steps(tt, pp):
            """V projection for token tile tt, head-pair pp (128 cols)."""
            vbase = 256 + 384 * pp
            box = {}

            def step(kc):
                def go():
                    if kc == 0:
                        box['pj'] = ps.tile([128, 512], F32, tag="pj",
                                            bufs=2, name="pj")
                    nc.tensor.matmul(
                        box['pj'][:, 0:128],
                        XTW[kc][:, tt * 128:(tt + 1) * 128],
                        WT[kc][:, vbase:vbase + 128],
                        start=(kc == 0), stop=(kc == 7))
                    if kc == 7:
                        pj = box['pj']
                        vv = V[tt].rearrange("p (h e) -> p h e", e=65)
                        nc.vector.tensor_copy(
                            vv[:, 2 * pp:2 * pp + 2, 0:64],
                            pj[:, 0:128].rearrange("p (h e) -> p h e", e=64))
                        nc.vector.memset(vv[:, 2 * pp:2 * pp + 2, 64:65], 1.0)
                return go
            return [step(kc) for kc in range(8)]

        def outproj_steps(tt):
            box = {}

            def step(fb, kcp):
                def go():
                    if fb == 0 and kcp == 0:
                        box['ysb'] = wk.tile([128, C], BF16, tag="ysb",
                                             bufs=2, name="ysb")
                    if kcp == 0:
                        box[fb] = ps.tile([128, 512], F32, tag="pj",
                                          bufs=2, name="pj")
                    nc.tensor.matmul(
                        box[fb][:],
                        OUTT[kcp][:, tt * 128:(tt + 1) * 128],
                        WOT[kcp][:, fb * 512:(fb + 1) * 512],
                        start=(kcp == 0), stop=(kcp == 1))
                    if kcp == 1:
                        nc.vector.tensor_copy(
                            box['ysb'][:, fb * 512:(fb + 1) * 512],
                            box[fb][:])
                    if fb == 1 and kcp == 1:
                        eng = nc.sync if tt % 2 else nc.gpsimd
                        eng.dma_start(y_d[tt * 128:(tt + 1) * 128, :],
                                      box['ysb'][:])
                return go
            return [step(fb, kcp) for fb in range(2) for kcp in range(2)]

        # Wide out-projection for the tail: once the attention PSUM banks
        # free up, each token tile gets a full [128,1024] tile (both fb
        # halves, no pj-buffer contention), the OUTT[0] contraction half
        # runs before the last normalize lands, and y DMAs alternate
        # queues.
        wbox = {}

        def wide_pre(tt):
            tag = "pv" if tt in (10, 13) else "st"
            w = ps.tile([128, 1024], F32, tag=tag,
                        bufs=(1 if tag == "pv" else 2), name=f"w{tt}")
            if tt >= 14:
                # separate half tiles: the ACT and DVE eviction copies run
                # concurrently (same-tile writes serialize across engines)
                ysb = (wk.tile([128, 512], BF16, tag="ysbwL", bufs=2,
                               name=f"ywL{tt}"),
                       wk.tile([128, 512], BF16, tag="ysbwR", bufs=2,
                               name=f"ywR{tt}"))
            else:
                ysb = wk.tile([128, C], BF16, tag="ysbw", bufs=4,
                              name=f"yw{tt}")
            wbox[tt] = (w, ysb)
            for fb in range(2):
                nc.tensor.matmul(
                    w[:, fb * 512:(fb + 1) * 512],
                    OUTT[0][:, tt * 128:(tt + 1) * 128],
                    WOT[0][:, fb * 512:(fb + 1) * 512],
                    start=True, stop=False)

        def wide_post(tt, split=False):
            w, ysb = wbox[tt]
            for fb in range(2):
                nc.tensor.matmul(
                    w[:, fb * 512:(fb + 1) * 512],
                    OUTT[1][:, tt * 128:(tt + 1) * 128],
                    WOT[1][:, fb * 512:(fb + 1) * 512],
                    start=False, stop=True)
            if isinstance(ysb, tuple):
                ysbL, ysbR = ysb
                nc.scalar.copy(ysbL[:], w[:, 0:512])
                nc.sync.dma_start(y_d[tt * 128:(tt + 1) * 128, 0:512],
                                  ysbL[:])
                nc.vector.tensor_copy(ysbR[:], w[:, 512:1024])
                nc.gpsimd.dma_start(y_d[tt * 128:(tt + 1) * 128, 512:1024],
                                    ysbR[:])
            else:
                nc.scalar.copy(ysb[:, 0:512], w[:, 0:512])
                nc.vector.tensor_copy(ysb[:, 512:1024], w[:, 512:1024])
                eng = nc.sync if tt % 2 else nc.gpsimd
                eng.dma_start(y_d[tt * 128:(tt + 1) * 128, :], ysb[:])

        def attention_qb(p, qb, filler=None, rate=0, boundary=0, final=False,
                         adapt=False):
            hA, hB = 2 * p, 2 * p + 1
            if filler is not None and boundary:
                filler.fill(boundary)
            # merged A/B psum: head A in cols 0:512, head B in 512:1024
            pv = ps.tile([128, 1024], F32, tag="pv", bufs=1, name="pv")
            nkt = (qb + 1) * 4

            def emit_pv(kt, sa, off, ncols):
                nc.tensor.matmul(
                    pv[0:65, off:512],
                    V[kt][:, hA * 65:hA * 65 + 65],
                    sa[:, 0:ncols],
                    start=(kt == 0), stop=(kt == nkt - 1))
                nc.tensor.matmul(
                    pv[0:65, 512 + off:1024],
                    V[kt][:, hB * 65:hB * 65 + 65],
                    sa[:, 512:512 + ncols],
                    start=(kt == 0), stop=(kt == nkt - 1))

            # software-pipelined: PV(kt) is emitted after ST(kt+1) so the
            # in-order PE never waits on the exp of the current iteration
            pending = None
            for kt in range(nkt):
                off = max(0, kt * 128 - qb * 512)
                ncols = 512 - off
                qs = qb * 512 + off
                stp = ps.tile([128, 1024], F32, tag="st", bufs=2, name="stp")
                nc.tensor.matmul(
                    stp[:, 0:ncols],
                    KT[p][0:64, kt * 128:(kt + 1) * 128],
                    QT[p][0:64, qs:qs + ncols],
                    start=True, stop=True)
                nc.tensor.matmul(
                    stp[:, 512:512 + ncols],
                    KT[p][64:128, kt * 128:(kt + 1) * 128],
                    QT[p][64:128, qs:qs + ncols],
                    start=True, stop=True)
                sa = wk.tile([128, 1024], BF16, tag="sa_sb", bufs=3, name="sa")
                nc.scalar.activation(
                    sa.rearrange("p (g n) -> p g n", g=2)[:, :, 0:ncols],
                    stp.rearrange("p (g n) -> p g n", g=2)[:, :, 0:ncols],
                    AF.Exp, bias=zbias[:], scale=0.125)
                if off > 0 or kt * 128 == qb * 512:
                    m3 = sa.rearrange("p (g n) -> p g n", g=2)[:, :, 0:128]
                    nc.vector.tensor_mul(
                        m3, m3,
                        trimask[:].unsqueeze(1).broadcast_to([128, 2, 128]))
                if filler is not None:
                    extra = (0 if ncols == 512 else
                             (1 if ncols >= 256 else 2))
                    filler.fill(rate + extra)
                if pending is not None:
                    emit_pv(*pending)
                pending = (kt, sa, off, ncols)
            emit_pv(*pending)
            # evict unnormalized out + Z; 1/Z via DMA-reshape so the
            # reciprocal runs on 128 DVE lanes instead of one
            if final:
                zr = wk.tile([1, 1024], F32, tag="zr", bufs=1, name="zr")
                nc.vector.tensor_copy(zr[:, 0:512], pv[64:65, 0:512])
                nc.scalar.copy(zr[:, 512:1024], pv[64:65, 512:1024])
            u = wk.tile([65, 1024], F32, tag="u", bufs=2, name="u")
            nc.vector.tensor_copy(u[:, 0:512], pv[0:65, 0:512])
            if final:
                nc.scalar.copy(u[:, 512:1024], pv[0:65, 512:1024])
            else:
                nc.vector.tensor_copy(u[:, 512:1024], pv[0:65, 512:1024])
            qsl = slice(qb * 512, (qb + 1) * 512)
            if final:
                # transposed 1/Z (reciprocal across DVE lanes) as two
                # independent half-chains on separate DMA queues, then two
                # 1-deep matmuls against a ones row replicate it across
                # partitions
                zcA = wk.tile([64, 8], F32, tag="zcA", bufs=1, name="zcA")
                zcB = wk.tile([64, 8], F32, tag="zcB", bufs=1, name="zcB")
                nc.sync.dma_start(zcA[:], zr[:, 0:512])
                nc.gpsimd.dma_start(zcB[:], zr[:, 512:1024])
                zcbA = wk.tile([64, 8], BF16, tag="zcbA", bufs=1, name="zcbA")
                zcbB = wk.tile([64, 8], BF16, tag="zcbB", bufs=1, name="zcbB")
                with nc.allow_low_precision(reason="bf16 1/Z, ~0.4% rel"):
                    nc.vector.reciprocal(zcbA[:], zcA[:])
                    nc.vector.reciprocal(zcbB[:], zcB[:])
                zrbA = wk.tile([1, 512], BF16, tag="zrbA", bufs=1, name="zrbA")
                zrbB = wk.tile([1, 512], BF16, tag="zrbB", bufs=1, name="zrbB")
                nc.sync.dma_start(zrbA[:], zcbA[:])
                nc.gpsimd.dma_start(zrbB[:], zcbB[:])
                # the chain above runs on DVE/ACT/sync/gpsimd; the PE
                # meanwhile drains the remaining filler reserve (qb<=1
                # out-projections) and runs the freed-bank wide
                # out-projections for qb2's tokens + the OUTT[0] halves of
                # qb3's
                if filler is not None:
                    filler.drain()
                for tt in (8, 9, 10, 11):
                    wide_pre(tt)
                    wide_post(tt)
                wide_pre(12)
                wide_pre(13)
                wide_pre(14)
                zpA = ps.tile([128, 512], F32, tag="pj", bufs=2, name="zpA")
                zpB = ps.tile([128, 512], F32, tag="pj", bufs=2, name="zpB")
                nc.tensor.matmul(zpA[0:64, :], onesb[:], zrbA[:],
                                 start=True, stop=True)
                nc.vector.tensor_mul(OUTT[p][0:64, qsl], u[0:64, 0:512],
                                     zpA[0:64, :])
                nc.tensor.matmul(zpB[0:64, :], onesb[:], zrbB[:],
                                 start=True, stop=True)
                nc.vector.tensor_mul(OUTT[p][64:128, qsl], u[0:64, 512:1024],
                                     zpB[0:64, :])
                wide_post(12)
                wide_post(13)
                wide_pre(15)
                wide_post(14, split=True)
                wide_post(15, split=True)
            else:
                zcol = wk.tile([128, 8], F32, tag="zcol", bufs=2, name="zcol")
                nc.sync.dma_start(zcol[:], u[64:65, :])
                nc.vector.reciprocal(zcol[:], zcol[:])
                zrow = wk.tile([1, 1024], F32, tag="zrow", bufs=2, name="zrow")
                nc.sync.dma_start(zrow[:], zcol[:])
                zb = wk.tile([64, 1024], F32, tag="zb", bufs=2, name="zb")
                nc.gpsimd.partition_broadcast(zb[:], zrow[:])
                nc.vector.tensor_mul(OUTT[p][0:64, qsl], u[0:64, 0:512],
                                     zb[:, 0:512])
                nc.vector.tensor_mul(OUTT[p][64:128, qsl], u[0:64, 512:1024],
                                     zb[:, 512:1024])

        # ---- emission order = PE order (in-order engine) ----
        # Pre-ramp: dummy matmuls with no data deps keep the PE busy from
        # the end of the framework preamble so it reaches full p-state
        # before real data lands, and pad the DMA-paced prefix so arrival
        # gaps never reset the ramp.
        stdum = ps.tile([128, 1024], F32, tag="st", bufs=2, name="stdum")

        def dummy_mm():
            nc.tensor.matmul(stdum[:, 0:256], dmy[:, 0:128], dmy[:],
                             start=True, stop=True, skip_group_check=True)

        for _ in range(8):
            dummy_mm()

        # Prefix phase A, paced to DMA: per k-chunk [Q0, K0, dummy, dummy]
        # so each 224KB chunk arrival feeds ~1.7us of PE work at mid
        # p-state. One accumulation group per PSUM bank (the PE start flag
        # zeroes bank-granular regions).
        pjQ = ps.tile([128, 512], F32, tag="pj", bufs=2, name="pjQ")
        pjK = ps.tile([128, 512], F32, tag="pj", bufs=2, name="pjK")
        pjV = ps.tile([128, 1024], F32, tag="pv", bufs=1, name="pjV")
        stV = ps.tile([128, 1024], F32, tag="st", bufs=2, name="stV")
        for kc in range(8):
            nc.tensor.matmul(pjQ[:], WT[kc][:, 0:128],
                             XTW[kc][:, 0:512],
                             start=(kc == 0), stop=(kc == 7))
            nc.tensor.matmul(pjK[:], WT[kc][:, 128:256],
                             XTW[kc][:, 0:512],
                             start=(kc == 0), stop=(kc == 7))
            nc.tensor.matmul(pjV[:, 0:128],
                             XTW[kc][:, 0:128], WT[kc][:, 256:384],
                             start=(kc == 0), stop=(kc == 7))
            nc.tensor.matmul(pjV[:, 512:640],
                             XTW[kc][:, 128:256], WT[kc][:, 256:384],
                             start=(kc == 0), stop=(kc == 7))
            nc.tensor.matmul(stV[:, 0:128],
                             XTW[kc][:, 256:384], WT[kc][:, 256:384],
                             start=(kc == 0), stop=(kc == 7))
            nc.tensor.matmul(stV[:, 512:640],
                             XTW[kc][:, 384:512], WT[kc][:, 256:384],
                             start=(kc == 0), stop=(kc == 7))
        nc.vector.tensor_copy(QT[0][:, 0:512], pjQ[:])
        nc.vector.tensor_copy(KT[0][:, 0:512], pjK[:])
        vdst = {0: pjV[:, 0:128], 1: pjV[:, 512:640],
                2: stV[:, 0:128], 3: stV[:, 512:640]}
        for tt in (0, 1, 2, 3):
            vv = V[tt].rearrange("p (h e) -> p h e", e=65)
            nc.vector.tensor_copy(
                vv[:, 0:2, 0:64],
                vdst[tt].rearrange("p (h e) -> p h e", e=64))
            nc.vector.memset(vv[:, 0:2, 64:65], 1.0)
        # release the dummy psum tile
        scratch = wk.tile([128, 8], F32, tag="scratch", bufs=1, name="scratch")
        nc.vector.tensor_copy(scratch[:], stdum[:, 0:8])

        # fillA: lead with work whose inputs are already resident (Q1/K1
        # nb0 needs only WT piece 2), then pair-0's deps in qb order, then
        # pair-1 early work
        fillA = Filler()
        markA = {}
        for w in range(2):
            fillA.add_group(proj_qk_steps(1, w, 0))
        for nb in (1, 2, 3):
            for w in range(2):
                fillA.add_group(proj_qk_steps(0, w, nb))
            m = 0
            for tt in range(4 * nb, 4 * nb + 4):
                m = fillA.add_group(proj_v_steps(tt, 0))
            markA[nb] = m
        for w in range(2):
            fillA.add_group(proj_qk_steps(1, w, 1))
        for tt in range(0, 8):
            fillA.add_group(proj_v_steps(tt, 1))

        for qb in range(4):
            if qb in markA:
                fillA.fill_until(markA[qb])
            attention_qb(0, qb, filler=fillA, rate=3,
                         boundary=(8 if qb >= 2 else 6 if qb else 0))
        fillA.drain()

        # fillB: pair-1 reserves ordered so every boundary has ready work,
        # out-projection groups one block late so OUTT[1] normalize chains
        # complete off-engine before their matmuls are reached
        fillB = Filler()
        for w in range(2):
            fillB.add_group(proj_qk_steps(1, w, 2))
        mB2 = 0
        for tt in range(8, 12):
            mB2 = fillB.add_group(proj_v_steps(tt, 1))
        for tt in range(0, 4):
            fillB.add_group(outproj_steps(tt))
        for w in range(2):
            fillB.add_group(proj_qk_steps(1, w, 3))
        mB3 = 0
        for tt in range(12, 16):
            mB3 = fillB.add_group(proj_v_steps(tt, 1))
        for tt in range(4, 8):
            fillB.add_group(outproj_steps(tt))

        attention_qb(1, 0, filler=fillB, rate=2, boundary=6)
        attention_qb(1, 1, filler=fillB, rate=2, boundary=6)
        fillB.fill_until(mB2)
        attention_qb(1, 2, filler=fillB, rate=3, boundary=6)
        fillB.fill_until(mB3)
        attention_qb(1, 3, filler=fillB, rate=1, boundary=4, final=True)
        fillB.drain()

    nc.compile()
    _cached_nc = nc
    return nc


def shard_inputs(x, Wqkv, Wout):
    """Full inputs -> 8 per-core input dicts (sliced/transposed/bf16-cast)."""
    bf = ml_dtypes.bfloat16
    Wq, Wk, Wv = Wqkv[0:1024], Wqkv[1024:2048], Wqkv[2048:3072]
    in_maps = []
    for i in range(N_CORES):
        b, g = divmod(i, 4)
        r0 = slice(256 * g, 256 * g + 128)
        r1 = slice(256 * g + 128, 256 * (g + 1))
        w_my = np.concatenate(
            [Wq[r0], Wk[r0], Wv[r0], Wq[r1], Wk[r1], Wv[r1]], axis=0)
        r = slice(256 * g, 256 * (g + 1))
        in_maps.append({
            "xt": np.ascontiguousarray(x[b].T).astype(bf),
            "wt": np.ascontiguousarray(w_my.T).astype(bf),
            "wot": np.ascontiguousarray(Wout[:, r].T).astype(bf),
        })
    return in_maps


def gather_output(results):
    """8 per-core partial y (bf16) -> full [2, T, C] f32 output."""
    y = np.zeros((2, T, C), dtype=np.float64)
    for i in range(N_CORES):
        y[i // 4] += np.asarray(results[i]["y"], dtype=np.float64)
    return y.astype(np.float32)


def kernel(x, Wqkv, Wout):
    x = np.asarray(x)
    Wqkv = np.asarray(Wqkv)
    Wout = np.asarray(Wout)
    nc = build_program()
    in_maps = shard_inputs(x, Wqkv, Wout)
    res = run_bass_kernel_spmd(nc, in_maps, core_ids=list(range(N_CORES)))
    return gather_output(res.results)

